# revision 7
# baseline (speedup 1.0000x reference)
"""Causal self-attention (B=2, L=4096, D=768, H=12) on 8 Trainium2 cores.

Sharding: core c = (b, g) with b = c // 4, g = c % 4. Data-parallel over the
batch, tensor-parallel over heads (3 heads per core). Each core computes its
heads' attention over the full sequence plus its slice of the output
projection (row-parallel); the host sums the 4 partial projections per batch
element and adds b_out.

Device-side design (all matmul operands bf16, fp32 PSUM accumulation):
  - host supplies x[b]^T (768, L); q^T/k^T are produced in [dh, L] layout by
    the projection itself, v in natural [L, dh] layout — no on-device
    transposes anywhere.
  - heads 0/1 are PACKED on partitions: qp/kp [128, L] hold head0 in rows
    0-63 and head1 in rows 64-127.  Their score matmuls are K=64 with
    tile_position row strips (0,0)/(64,0) and run CONCURRENTLY on the PE
    (two 64-row sub-arrays), halving score time for those heads.  Head 2
    (q2/k2, rows 0-63 only) runs K=64 solo on strips 0-1, overlapping the
    tail of head1's stream.
  - the head-0/1 score pair lands in one 2-bank-wide PSUM tile, so a SINGLE
    ScalarE exp covers 1024 columns — activation instruction count (and the
    per-instruction (112 cyc)/semaphore overhead on the bottleneck ScalarE)
    drops ~40%.  Scores are computed transposed, S^T[lk, lq], 512 lq per
    chunk; exp runs with the 1/sqrt(dh) scale folded in and no
    max-subtraction (scores are ~N(0,1) here, exp is safe in fp32).
  - causal mask = multiply by a 0/1 tile, needed only on the 4 diagonal
    lk-tiles of each lq chunk.
  - PV matmul uses lhsT = [v | ones] so PSUM row 64 accumulates the softmax
    denominator for free; the reciprocal row is partition-broadcast on the
    (otherwise idle) GpSimd engine and applied by one DVE multiply.
  - PSUM budget (8 banks): score-pair wide tile 2 + head-2 single 1 +
    pv accumulators 3 + filler (qkv/out-proj) pool 2.
  - the PE clock-gate (HAM) only sustains 2.4 GHz when the PE never idles,
    so the qkv projection of the NEXT lq chunk and the output projection of
    the PREVIOUS chunk are emitted as filler work inside the attention
    loop, and PV runs software-pipelined one 2-lk-tile group behind the exp.
"""

import os
import sys

sys.path.insert(0, "/opt/trn_rl_repo")

import numpy as np
import ml_dtypes

import concourse.bass as bass  # noqa: F401  (registers AP machinery)
import concourse.mybir as mybir
from concourse import bacc
import concourse.tile as tile
from concourse.bass_utils import run_bass_kernel_spmd

BF16 = ml_dtypes.bfloat16
F32 = mybir.dt.float32
BF = mybir.dt.bfloat16

D_MODEL = 768
N_HEADS = 12
D_HEAD = 64
B = 2
L_FULL = 4096
N_CORES = 8
TPG = 4  # head-groups (tensor-parallel degree per batch element)
HPC = N_HEADS // TPG  # 3 heads per core
DG = HPC * D_HEAD  # 192 feature dims per core
SCALE = 1.0 / np.sqrt(D_HEAD)

DM_CHUNKS = D_MODEL // 128  # 6


def build_nc(L=L_FULL):
    """Build the per-core Bass program (same program for all 8 cores)."""
    LC = L // 512  # lq chunks
    LT = L // 128  # lk / l tiles
    nc = bacc.Bacc("TRN2", target_bir_lowering=False, debug=False,
                   num_devices=N_CORES)

    xT_d = nc.dram_tensor("xT", [D_MODEL, L], BF, kind="ExternalInput").ap()
    wqk_d = nc.dram_tensor("wqkc", [D_MODEL, 384], BF, kind="ExternalInput").ap()
    bqk_d = nc.dram_tensor("bqkt", [128, 3], F32, kind="ExternalInput").ap()
    wv_d = nc.dram_tensor("wv", [D_MODEL, DG], BF, kind="ExternalInput").ap()
    wo2_d = nc.dram_tensor("wo2", [128, D_MODEL], BF, kind="ExternalInput").ap()
    wo3_d = nc.dram_tensor("wo3", [128, D_MODEL], BF, kind="ExternalInput").ap()
    mask_d = nc.dram_tensor("masks", [128, 4, 512], BF, kind="ExternalInput").ap()
    out_d = nc.dram_tensor("out", [L, D_MODEL], F32, kind="ExternalOutput").ap()

    with tile.TileContext(nc) as tc:
        with tc.tile_pool(name="persist", bufs=1) as persist:
            xT_sb = persist.tile([128, DM_CHUNKS, L], BF)
            wqk_sb = persist.tile([128, DM_CHUNKS, 384], BF)
            wv_sb = persist.tile([128, DM_CHUNKS, DG], BF)
            bqk_sb = persist.tile([128, 3], F32)
            wo2_sb = persist.tile([128, D_MODEL], BF)
            wo3_sb = persist.tile([128, D_MODEL], BF)
            mask_sb = persist.tile([128, 4, 512], BF)
            # heads 0/1 packed on partitions (rows 0-63 / 64-127); head 2
            # lives alone in rows 0-63 of its own tiles -> K=64 matmuls, no
            # zero padding anywhere
            qp = persist.tile([128, L], BF, name="qp01")
            kp = persist.tile([128, L], BF, name="kp01")
            q2 = persist.tile([64, L], BF, name="q2")
            k2 = persist.tile([64, L], BF, name="k2")
            vones = persist.tile([128, LT, HPC * 65], BF)
            attnT01 = persist.tile([128, L], BF)
            attnT2 = persist.tile([128, L], BF)

            # warmup scratch: garbage matmuls during the initial DMA keep the
            # PE busy so the HAM clock-gate is already at 2.4 GHz when real
            # work arrives
            scratch = persist.tile([128, 512], BF, name="scratch")

            # only the interleaved ones-columns of vones need initialising
            # (the data columns are fully written by the v evacuations); the
            # proj reads attnT2 rows 64-127 as zero K-padding
            nc.vector.memset(scratch, 0.0)
            nc.vector.memset(vones[:, :, 64:HPC * 65:65], 1.0)
            nc.gpsimd.memset(attnT2[64:128, :], 0.0)
            # x^T in lq-ordered strips so chunk 0's projections start early;
            # pair each weight chunk with its 512-wide first strip so fc=0's
            # accumulation chain unblocks as soon as possible
            strips = [(0, 512), (512, 1024)]
            strips += [(s, s + 1024) for s in range(1024, L, 1024)]
            for cdm in range(DM_CHUNKS):
                nc.sync.dma_start(out=wqk_sb[:, cdm, :],
                                  in_=wqk_d[cdm * 128:(cdm + 1) * 128, :])
                nc.sync.dma_start(
                    out=xT_sb[:, cdm, 0:512],
                    in_=xT_d[cdm * 128:(cdm + 1) * 128, 0:512])
            nc.sync.dma_start(out=mask_sb, in_=mask_d)
            nc.sync.dma_start(out=bqk_sb, in_=bqk_d)
            for s0, s1 in strips[1:2]:
                for cdm in range(DM_CHUNKS):
                    nc.sync.dma_start(
                        out=xT_sb[:, cdm, s0:s1],
                        in_=xT_d[cdm * 128:(cdm + 1) * 128, s0:s1])
            for cdm in range(DM_CHUNKS):
                nc.sync.dma_start(out=wv_sb[:, cdm, :],
                                  in_=wv_d[cdm * 128:(cdm + 1) * 128, :])
            nc.sync.dma_start(out=wo2_sb, in_=wo2_d)
            nc.sync.dma_start(out=wo3_sb, in_=wo3_d)
            for s0, s1 in strips[2:]:
                for cdm in range(DM_CHUNKS):
                    nc.sync.dma_start(
                        out=xT_sb[:, cdm, s0:s1],
                        in_=xT_d[cdm * 128:(cdm + 1) * 128, s0:s1])

            # wqkc column chunks: 0=[q0|q1] 1=[k0|k1] 2=[q2|k2]; chunks 0/1
            # evacuate whole (partition layout already matches qp/kp), chunk
            # 2 splits rows 0-63 -> q2, rows 64-127 -> k2 (shifted copy)
            with (
                tc.tile_pool(name="p1psum", bufs=3, space="PSUM") as p1p,
                tc.tile_pool(name="wpsum", bufs=1, space="PSUM") as wp,
                tc.tile_pool(name="pvpsum", bufs=3, space="PSUM") as pvp,
                tc.tile_pool(name="ptwpool", bufs=3) as ptwp,
                tc.tile_pool(name="ptspool", bufs=3) as ptsp,
                tc.tile_pool(name="rpool", bufs=3) as rp,
                tc.tile_pool(name="outpool", bufs=3) as outp,
            ):
                # PE warmup during the input DMA (see scratch above)
                for i in range(26):
                    pw = p1p.tile([128, 512], F32, tag="p1", name=f"warm{i}")
                    nc.tensor.matmul(pw, scratch[:, 0:128], scratch,
                                     start=True, stop=True)
                def emit_qk(fc, lc):
                    ps = p1p.tile([128, 512], F32, tag="p1",
                                  name=f"psqk{fc}_{lc}")
                    for cdm in range(DM_CHUNKS):
                        nc.tensor.matmul(
                            ps,
                            wqk_sb[:, cdm, fc * 128:(fc + 1) * 128],
                            xT_sb[:, cdm, lc * 512:(lc + 1) * 512],
                            start=(cdm == 0), stop=(cdm == DM_CHUNKS - 1),
                        )
                    lq = slice(lc * 512, (lc + 1) * 512)
                    if fc == 0:
                        nc.vector.tensor_scalar_add(
                            qp[:, lq], ps, bqk_sb[:, 0:1])
                    elif fc == 1:
                        nc.vector.tensor_scalar_add(
                            kp[:, lq], ps, bqk_sb[:, 1:2])
                    else:
                        nc.vector.tensor_scalar_add(
                            q2[0:64, lq], ps[0:64, :], bqk_sb[0:64, 2:3])
                        nc.vector.tensor_scalar_add(
                            k2[0:64, lq], ps[64:128, :], bqk_sb[64:128, 2:3])

                def emit_v(lt):
                    ps = p1p.tile([128, DG], F32, tag="p1", name=f"psv{lt}")
                    for cdm in range(DM_CHUNKS):
                        nc.tensor.matmul(
                            ps,
                            xT_sb[:, cdm, lt * 128:(lt + 1) * 128],
                            wv_sb[:, cdm, :],
                            start=(cdm == 0), stop=(cdm == DM_CHUNKS - 1),
                        )
                    nc.vector.tensor_copy(
                        vones[:, lt, 0:HPC * 65]
                        .rearrange("p (h c) -> p h c", h=HPC)[:, :, 0:64],
                        ps.rearrange("p (h c) -> p h c", h=HPC),
                    )

                def emit_proj(lt):
                    osb = outp.tile([128, D_MODEL], F32, tag="osb",
                                    name=f"osb{lt}")
                    for nh in range(2):
                        po = p1p.tile([128, 384], F32, tag="p1",
                                      name=f"po{lt}_{nh}")
                        nc.tensor.matmul(
                            po,
                            attnT01[:, lt * 128:(lt + 1) * 128],
                            wo2_sb[:, nh * 384:(nh + 1) * 384],
                            start=True, stop=False,
                        )
                        nc.tensor.matmul(
                            po,
                            attnT2[:, lt * 128:(lt + 1) * 128],
                            wo3_sb[:, nh * 384:(nh + 1) * 384],
                            start=False, stop=True,
                        )
                        nc.vector.tensor_copy(osb[:, nh * 384:(nh + 1) * 384], po)
                    nc.sync.dma_start(out=out_d[lt * 128:(lt + 1) * 128, :],
                                      in_=osb)

                def qkv_fillers(lc):
                    fs = [lambda fc=fc: emit_qk(fc, lc) for fc in range(3)]
                    fs += [lambda lt=lt: emit_v(lt)
                           for lt in range(4 * lc, 4 * lc + 4)]
                    return fs

                for f in qkv_fillers(0):
                    f()

                def emit_scores(c, t):
                    """Head-0/1 pair (concurrent K=64 row strips, one 2-bank
                    PSUM tile, one 1024-wide exp) + head-2 solo (K=64, PSUM
                    from the filler pool) for lk tile t.  Scalar-queue order
                    (wide exp first, then head-2) matches the PE-side gates:
                    the single wide buffer is re-written only two Scalar
                    instructions after its exp, and the filler pool's
                    rotation distance of 3 covers the head-2 tiles."""
                    j = t - 4 * c
                    col0 = 128 * j if j >= 0 else 0
                    lq = slice(c * 512 + col0, (c + 1) * 512)
                    tc_ = slice(t * 128, (t + 1) * 128)
                    w = wp.tile([128, 2, 512], F32, tag="w",
                                name=f"w_c{c}t{t}")
                    nc.tensor.matmul(w[:, 0, col0:], kp[0:64, tc_],
                                     qp[0:64, lq])
                    nc.tensor.matmul(w[:, 1, col0:], kp[64:128, tc_],
                                     qp[64:128, lq])
                    ptw = ptwp.tile([128, 2, 512], BF, tag="ptw",
                                    name=f"ptw_c{c}t{t}")
                    nc.scalar.activation(
                        ptw[:, :, col0:], w[:, :, col0:],
                        mybir.ActivationFunctionType.Exp, scale=float(SCALE))
                    s = p1p.tile([128, 512], F32, tag="p1", name=f"s_c{c}t{t}")
                    nc.tensor.matmul(s[:, col0:], k2[0:64, tc_],
                                     q2[0:64, lq])
                    pts = ptsp.tile([128, 512], BF, tag="pts",
                                    name=f"pts_c{c}t{t}")
                    nc.scalar.activation(
                        pts[:, col0:], s[:, col0:],
                        mybir.ActivationFunctionType.Exp, scale=float(SCALE))
                    if j >= 0:
                        # only the 128-wide diagonal band is partial;
                        # columns beyond it are fully valid
                        band = slice(col0, col0 + 128)
                        mb = mask_sb[:, j, band]
                        nc.vector.tensor_mul(ptw[:, 0, band], ptw[:, 0, band], mb)
                        nc.vector.tensor_mul(ptw[:, 1, band], ptw[:, 1, band], mb)
                        nc.vector.tensor_mul(pts[:, band], pts[:, band], mb)
                    return [(0, ptw[:, 0, :], col0, t),
                            (1, ptw[:, 1, :], col0, t),
                            (2, pts, col0, t)]

                # (attnT tile, destination row base) per head
                norm_dest = [(attnT01, 0), (attnT01, 64), (attnT2, 0)]
                for c in range(LC):
                    nt = 4 * (c + 1)
                    ngroups = nt // 2
                    fillers = qkv_fillers(c + 1) if c + 1 < LC else []
                    if c >= 1:
                        fillers += [lambda lt=lt: emit_proj(lt)
                                    for lt in range(4 * (c - 1), 4 * c)]
                    pv_acc = [pvp.tile([65, 512], F32, tag="pvacc",
                                       name=f"pvacc_c{c}h{h}")
                              for h in range(HPC)]
                    prev = []
                    fi = 0
                    for g in range(ngroups):
                        cur = emit_scores(c, 2 * g)
                        # some PE filler work between the two score tiles
                        # bridges the wide exp of tile t draining before the
                        # single wide PSUM buffer is re-written for t+1
                        want = (2 * g + 1) * len(fillers) // (2 * ngroups)
                        while fi < want:
                            fillers[fi]()
                            fi += 1
                        cur += emit_scores(c, 2 * g + 1)
                        # software-pipelined PV: one 2-tile group behind
                        for (h, src, c0, t0) in prev:
                            nc.tensor.matmul(
                                pv_acc[h][:, c0:],
                                vones[:, t0, h * 65:(h + 1) * 65],
                                src[:, c0:],
                                start=(t0 == 0), stop=False,
                            )
                        prev = cur
                        want = (g + 1) * len(fillers) // ngroups
                        while fi < want:
                            fillers[fi]()
                            fi += 1
                    for (h, src, c0, t0) in prev:
                        nc.tensor.matmul(
                            pv_acc[h][:, c0:],
                            vones[:, t0, h * 65:(h + 1) * 65],
                            src[:, c0:],
                            start=(t0 == 0), stop=(t0 == nt - 1),
                        )
                    rbs_h = []
                    for h in range(HPC):
                        dn = rp.tile([1, 512], F32, tag="dn",
                                     name=f"dn_c{c}h{h}")
                        # partition-shifting copy (psum row 64 -> sbuf row 0);
                        # partition_broadcast only honors a partition-0 source
                        nc.vector.tensor_copy(dn[0:1, :],
                                              pv_acc[h][64:65, :])
                        dnb = rp.tile([64, 512], F32, tag="dnb",
                                      name=f"dnb_c{c}h{h}")
                        nc.gpsimd.partition_broadcast(dnb, dn[0:1, :])
                        rbs = rp.tile([64, 512], F32, tag="rbs",
                                      name=f"rbs_c{c}h{h}")
                        nc.vector.reciprocal_approx_fast(out=rbs, in_=dnb)
                        rbs_h.append(rbs)
                    if c < LC - 1:
                        for h in range(HPC):
                            dt_, r0 = norm_dest[h]
                            nc.vector.tensor_mul(
                                dt_[r0:r0 + 64, c * 512:(c + 1) * 512],
                                pv_acc[h][0:64, :], rbs_h[h],
                            )
                    else:
                        # last chunk: normalise per 128-column piece and kick
                        # its output projection immediately so the epilogue
                        # overlaps DVE/PE/DMA instead of serialising
                        for i in range(4):
                            pc = slice(128 * i, 128 * (i + 1))
                            for h in range(HPC):
                                dt_, r0 = norm_dest[h]
                                nc.vector.tensor_mul(
                                    dt_[r0:r0 + 64,
                                        c * 512 + 128 * i:c * 512 + 128 * (i + 1)],
                                    pv_acc[h][0:64, pc], rbs_h[h][:, pc],
                                )
                            emit_proj(4 * c + i)

    nc.compile()
    return nc


def make_in_maps(x, w_qkv, b_qkv, w_out, L=L_FULL):
    """Host-side sharding: build the 8 per-core input dicts."""
    # causal mask tiles for diagonal blocks: m[p, j, f] = (128 j + p) <= f
    p = np.arange(128)[:, None, None]
    jj = np.arange(4)[None, :, None]
    f = np.arange(512)[None, None, :]
    masks = ((128 * jj + p) <= f).astype(BF16)

    xT = [np.ascontiguousarray(x[b].T.astype(BF16)) for b in range(B)]
    in_maps = []
    for c in range(N_CORES):
        b, g = divmod(c, TPG)
        h0 = g * HPC  # first global head of this group

        def qcol(h):
            return slice((h0 + h) * D_HEAD, (h0 + h + 1) * D_HEAD)

        def kcol(h):
            return slice(768 + (h0 + h) * D_HEAD, 768 + (h0 + h + 1) * D_HEAD)

        wqkc = np.zeros((D_MODEL, 384), np.float32)
        bqkc = np.zeros((384,), np.float32)
        # chunk0 [q0|q1], chunk1 [k0|k1], chunk2 [q2|k2]
        for h in range(2):
            wqkc[:, h * 64:(h + 1) * 64] = w_qkv[:, qcol(h)]
            wqkc[:, 128 + h * 64:128 + (h + 1) * 64] = w_qkv[:, kcol(h)]
            bqkc[h * 64:(h + 1) * 64] = b_qkv[qcol(h)]
            bqkc[128 + h * 64:128 + (h + 1) * 64] = b_qkv[kcol(h)]
        wqkc[:, 256:320] = w_qkv[:, qcol(2)]
        bqkc[256:320] = b_qkv[qcol(2)]
        wqkc[:, 320:384] = w_qkv[:, kcol(2)]
        bqkc[320:384] = b_qkv[kcol(2)]

        wv = w_qkv[:, 1536 + h0 * 64:1536 + (h0 + HPC) * 64]
        wo = w_out[h0 * 64:(h0 + HPC) * 64, :]
        wo3 = np.zeros((128, D_MODEL), np.float32)
        wo3[0:64] = wo[128:192]

        in_maps.append({
            "xT": xT[b][:, :L],
            "wqkc": wqkc.astype(BF16),
            "bqkt": np.ascontiguousarray(bqkc.reshape(3, 128).T),
            "wv": np.ascontiguousarray(wv).astype(BF16),
            "wo2": np.ascontiguousarray(wo[0:128]).astype(BF16),
            "wo3": wo3.astype(BF16),
            "masks": masks,
        })
    return in_maps


_NC_CACHE = {}


def _get_nc(L=L_FULL):
    if L not in _NC_CACHE:
        _NC_CACHE[L] = build_nc(L)
    return _NC_CACHE[L]


def run(x, w_qkv, b_qkv, w_out, b_out, L=L_FULL, trace=False):
    nc = _get_nc(L)
    in_maps = make_in_maps(np.asarray(x), np.asarray(w_qkv),
                           np.asarray(b_qkv), np.asarray(w_out), L=L)
    if trace:
        install_ntff()
    res = run_bass_kernel_spmd(nc, in_maps, core_ids=list(range(N_CORES)),
                               trace=trace)
    partials = np.stack([res.results[c]["out"] for c in range(N_CORES)])
    out = partials.reshape(B, TPG, L, D_MODEL).sum(axis=1)
    # the V bias commutes through the attention average (weights sum to 1),
    # so it collapses to a constant row applied after the projection
    bias = np.asarray(b_qkv, np.float32)[1536:] @ np.asarray(w_out, np.float32)
    out = out + (bias + np.asarray(b_out, np.float32))[None, None, :]
    return out.astype(np.float32), res


def kernel(x, w_qkv, b_qkv, w_out, b_out):
    out, _ = run(x, w_qkv, b_qkv, w_out, b_out, L=L_FULL, trace=False)
    return out


# ---- optional NTFF profiling hook (axon images lack antenv.axon_hooks) ----
def install_ntff(so_path="/opt/axon/libaxon_pjrt.so"):
    import contextlib
    import ctypes
    import types

    if "antenv.axon_hooks" in sys.modules:
        return
    holder = {"hook": None}

    def _build():
        if not os.path.exists(so_path):
            return None
        lib = ctypes.CDLL(so_path)
        if not hasattr(lib, "axon_start_nrt_profile"):
            return None
        lib.axon_start_nrt_profile.argtypes = [ctypes.POINTER(ctypes.c_int64),
                                               ctypes.c_size_t]
        lib.axon_start_nrt_profile.restype = ctypes.c_int64
        lib.axon_stop_nrt_profile.argtypes = [ctypes.c_char_p]
        lib.axon_stop_nrt_profile.restype = ctypes.c_int64

        @contextlib.contextmanager
        def _hook(output_dir, device_ids):
            import jax
            jax.devices()
            if device_ids:
                ids = (ctypes.c_int64 * len(device_ids))(*device_ids)
                rc = lib.axon_start_nrt_profile(ids, len(device_ids))
            else:
                rc = lib.axon_start_nrt_profile(None, 0)
            if rc != 0:
                raise RuntimeError(f"axon_start_nrt_profile rc={rc}")
            try:
                yield
            finally:
                n = lib.axon_stop_nrt_profile(str(output_dir).encode())
                print(f"ntff profile: {n} file(s) -> {output_dir}",
                      file=sys.stderr)

        return _hook

    mod = types.ModuleType("antenv.axon_hooks")
    mod.set_axon_ntff_profile_hook = lambda h: holder.__setitem__("hook", h)
    mod.get_axon_ntff_profile_hook = lambda: holder["hook"]
    sys.modules["antenv.axon_hooks"] = mod
    holder["hook"] = _build()


# revision 9
# speedup vs baseline: 1.0066x; 1.0066x over previous
"""Causal self-attention (B=2, L=4096, D=768, H=12) on 8 Trainium2 cores.

Sharding: core c = (b, g) with b = c // 4, g = c % 4. Data-parallel over the
batch, tensor-parallel over heads (3 heads per core). Each core computes its
heads' attention over the full sequence plus its slice of the output
projection (row-parallel); the host sums the 4 partial projections per batch
element and adds b_out.

Device-side design (all matmul operands bf16, fp32 PSUM accumulation):
  - host supplies x[b]^T (768, L); q^T/k^T are produced in [dh, L] layout by
    the projection itself, v in natural [L, dh] layout — no on-device
    transposes anywhere.
  - heads 0/1 are PACKED on partitions: qp/kp [128, L] hold head0 in rows
    0-63 and head1 in rows 64-127.  Their score matmuls are K=64 with
    tile_position row strips (0,0)/(64,0) and run CONCURRENTLY on the PE
    (two 64-row sub-arrays), halving score time for those heads.  Head 2
    (q2/k2, rows 0-63 only) runs K=64 solo on strips 0-1, overlapping the
    tail of head1's stream.
  - the head-0/1 score pair lands in one 2-bank-wide PSUM tile, so a SINGLE
    ScalarE exp covers 1024 columns — activation instruction count (and the
    per-instruction (112 cyc)/semaphore overhead on the bottleneck ScalarE)
    drops ~40%.  Scores are computed transposed, S^T[lk, lq], 512 lq per
    chunk; exp runs with the 1/sqrt(dh) scale folded in and no
    max-subtraction (scores are ~N(0,1) here, exp is safe in fp32).
  - causal mask = multiply by a 0/1 tile, needed only on the 4 diagonal
    lk-tiles of each lq chunk.
  - PV matmul uses lhsT = [v | ones] so PSUM row 64 accumulates the softmax
    denominator for free; the reciprocal row is partition-broadcast on the
    (otherwise idle) GpSimd engine and applied by one DVE multiply.
  - PSUM budget (8 banks): score-pair wide tile 2 + head-2 single 1 +
    pv accumulators 3 + filler (qkv/out-proj) pool 2.
  - the PE clock-gate (HAM) only sustains 2.4 GHz when the PE never idles,
    so the qkv projection of the NEXT lq chunk and the output projection of
    the PREVIOUS chunk are emitted as filler work inside the attention
    loop, and PV runs software-pipelined one 2-lk-tile group behind the exp.
"""

import os
import sys

sys.path.insert(0, "/opt/trn_rl_repo")

import numpy as np
import ml_dtypes

import concourse.bass as bass  # noqa: F401  (registers AP machinery)
import concourse.mybir as mybir
from concourse import bacc
import concourse.tile as tile
from concourse.bass_utils import run_bass_kernel_spmd

BF16 = ml_dtypes.bfloat16
F32 = mybir.dt.float32
BF = mybir.dt.bfloat16

D_MODEL = 768
N_HEADS = 12
D_HEAD = 64
B = 2
L_FULL = 4096
N_CORES = 8
TPG = 4  # head-groups (tensor-parallel degree per batch element)
HPC = N_HEADS // TPG  # 3 heads per core
DG = HPC * D_HEAD  # 192 feature dims per core
SCALE = 1.0 / np.sqrt(D_HEAD)

DM_CHUNKS = D_MODEL // 128  # 6


def build_nc(L=L_FULL):
    """Build the per-core Bass program (same program for all 8 cores)."""
    LC = L // 512  # lq chunks
    LT = L // 128  # lk / l tiles
    nc = bacc.Bacc("TRN2", target_bir_lowering=False, debug=False,
                   num_devices=N_CORES)

    xT_d = nc.dram_tensor("xT", [D_MODEL, L], BF, kind="ExternalInput").ap()
    wqk_d = nc.dram_tensor("wqkc", [D_MODEL, 384], BF, kind="ExternalInput").ap()
    bqk_d = nc.dram_tensor("bqkt", [128, 3], F32, kind="ExternalInput").ap()
    wv_d = nc.dram_tensor("wv", [D_MODEL, DG], BF, kind="ExternalInput").ap()
    wo2_d = nc.dram_tensor("wo2", [128, D_MODEL], BF, kind="ExternalInput").ap()
    wo3_d = nc.dram_tensor("wo3", [128, D_MODEL], BF, kind="ExternalInput").ap()
    mask_d = nc.dram_tensor("masks", [128, 4, 512], BF, kind="ExternalInput").ap()
    out_d = nc.dram_tensor("out", [L, D_MODEL], F32, kind="ExternalOutput").ap()

    with tile.TileContext(nc) as tc:
        with tc.tile_pool(name="persist", bufs=1) as persist:
            xT_sb = persist.tile([128, DM_CHUNKS, L], BF)
            wqk_sb = persist.tile([128, DM_CHUNKS, 384], BF)
            wv_sb = persist.tile([128, DM_CHUNKS, DG], BF)
            bqk_sb = persist.tile([128, 3], F32)
            wo2_sb = persist.tile([128, D_MODEL], BF)
            wo3_sb = persist.tile([128, D_MODEL], BF)
            mask_sb = persist.tile([128, 4, 512], BF)
            # heads 0/1 packed on partitions (rows 0-63 / 64-127); head 2
            # lives alone in rows 0-63 of its own tiles -> K=64 matmuls, no
            # zero padding anywhere
            qp = persist.tile([128, L], BF, name="qp01")
            kp = persist.tile([128, L], BF, name="kp01")
            q2 = persist.tile([64, L], BF, name="q2")
            k2 = persist.tile([64, L], BF, name="k2")
            vones = persist.tile([128, LT, HPC * 65], BF)
            attnT01 = persist.tile([128, L], BF)
            attnT2 = persist.tile([128, L], BF)

            # warmup scratch: garbage matmuls during the initial DMA keep the
            # PE busy so the HAM clock-gate is already at 2.4 GHz when real
            # work arrives
            scratch = persist.tile([128, 512], BF, name="scratch")

            # only the interleaved ones-columns of vones need initialising
            # (the data columns are fully written by the v evacuations); the
            # proj reads attnT2 rows 64-127 as zero K-padding
            nc.vector.memset(scratch, 0.0)
            nc.vector.memset(vones[:, :, 64:HPC * 65:65], 1.0)
            nc.gpsimd.memset(attnT2[64:128, :], 0.0)
            # x^T in lq-ordered strips so chunk 0's projections start early;
            # pair each weight chunk with its 512-wide first strip so fc=0's
            # accumulation chain unblocks as soon as possible
            strips = [(0, 512), (512, 1024)]
            strips += [(s, s + 1024) for s in range(1024, L, 1024)]
            for cdm in range(DM_CHUNKS):
                nc.sync.dma_start(out=wqk_sb[:, cdm, :],
                                  in_=wqk_d[cdm * 128:(cdm + 1) * 128, :])
                nc.sync.dma_start(
                    out=xT_sb[:, cdm, 0:512],
                    in_=xT_d[cdm * 128:(cdm + 1) * 128, 0:512])
            nc.sync.dma_start(out=mask_sb, in_=mask_d)
            nc.sync.dma_start(out=bqk_sb, in_=bqk_d)
            for s0, s1 in strips[1:2]:
                for cdm in range(DM_CHUNKS):
                    nc.sync.dma_start(
                        out=xT_sb[:, cdm, s0:s1],
                        in_=xT_d[cdm * 128:(cdm + 1) * 128, s0:s1])
            for cdm in range(DM_CHUNKS):
                nc.sync.dma_start(out=wv_sb[:, cdm, :],
                                  in_=wv_d[cdm * 128:(cdm + 1) * 128, :])
            nc.sync.dma_start(out=wo2_sb, in_=wo2_d)
            nc.sync.dma_start(out=wo3_sb, in_=wo3_d)
            for s0, s1 in strips[2:]:
                for cdm in range(DM_CHUNKS):
                    nc.sync.dma_start(
                        out=xT_sb[:, cdm, s0:s1],
                        in_=xT_d[cdm * 128:(cdm + 1) * 128, s0:s1])

            # wqkc column chunks: 0=[q0|q1] 1=[k0|k1] 2=[q2|k2]; chunks 0/1
            # evacuate whole (partition layout already matches qp/kp), chunk
            # 2 splits rows 0-63 -> q2, rows 64-127 -> k2 (shifted copy)
            with (
                tc.tile_pool(name="p1psum", bufs=3, space="PSUM") as p1p,
                tc.tile_pool(name="wpsum", bufs=1, space="PSUM") as wp,
                tc.tile_pool(name="pvpsum", bufs=3, space="PSUM") as pvp,
                tc.tile_pool(name="ptwpool", bufs=8) as ptwp,
                tc.tile_pool(name="ptspool", bufs=8) as ptsp,
                tc.tile_pool(name="rpool", bufs=3) as rp,
                tc.tile_pool(name="outpool", bufs=3) as outp,
            ):
                # PE warmup during the input DMA (see scratch above)
                for i in range(26):
                    pw = p1p.tile([128, 512], F32, tag="p1", name=f"warm{i}")
                    nc.tensor.matmul(pw, scratch[:, 0:128], scratch,
                                     start=True, stop=True)
                def emit_qk(fc, lc):
                    ps = p1p.tile([128, 512], F32, tag="p1",
                                  name=f"psqk{fc}_{lc}")
                    for cdm in range(DM_CHUNKS):
                        nc.tensor.matmul(
                            ps,
                            wqk_sb[:, cdm, fc * 128:(fc + 1) * 128],
                            xT_sb[:, cdm, lc * 512:(lc + 1) * 512],
                            start=(cdm == 0), stop=(cdm == DM_CHUNKS - 1),
                        )
                    lq = slice(lc * 512, (lc + 1) * 512)
                    if fc == 0:
                        nc.vector.tensor_scalar_add(
                            qp[:, lq], ps, bqk_sb[:, 0:1])
                    elif fc == 1:
                        nc.vector.tensor_scalar_add(
                            kp[:, lq], ps, bqk_sb[:, 1:2])
                    else:
                        nc.vector.tensor_scalar_add(
                            q2[0:64, lq], ps[0:64, :], bqk_sb[0:64, 2:3])
                        nc.vector.tensor_scalar_add(
                            k2[0:64, lq], ps[64:128, :], bqk_sb[64:128, 2:3])

                def emit_v(lt):
                    ps = p1p.tile([128, DG], F32, tag="p1", name=f"psv{lt}")
                    for cdm in range(DM_CHUNKS):
                        nc.tensor.matmul(
                            ps,
                            xT_sb[:, cdm, lt * 128:(lt + 1) * 128],
                            wv_sb[:, cdm, :],
                            start=(cdm == 0), stop=(cdm == DM_CHUNKS - 1),
                        )
                    nc.vector.tensor_copy(
                        vones[:, lt, 0:HPC * 65]
                        .rearrange("p (h c) -> p h c", h=HPC)[:, :, 0:64],
                        ps.rearrange("p (h c) -> p h c", h=HPC),
                    )

                def emit_proj(lt):
                    osb = outp.tile([128, D_MODEL], F32, tag="osb",
                                    name=f"osb{lt}")
                    for nh in range(2):
                        po = p1p.tile([128, 384], F32, tag="p1",
                                      name=f"po{lt}_{nh}")
                        nc.tensor.matmul(
                            po,
                            attnT01[:, lt * 128:(lt + 1) * 128],
                            wo2_sb[:, nh * 384:(nh + 1) * 384],
                            start=True, stop=False,
                        )
                        nc.tensor.matmul(
                            po,
                            attnT2[:, lt * 128:(lt + 1) * 128],
                            wo3_sb[:, nh * 384:(nh + 1) * 384],
                            start=False, stop=True,
                        )
                        nc.vector.tensor_copy(osb[:, nh * 384:(nh + 1) * 384], po)
                    nc.sync.dma_start(out=out_d[lt * 128:(lt + 1) * 128, :],
                                      in_=osb)

                def qkv_fillers(lc):
                    fs = [lambda fc=fc: emit_qk(fc, lc) for fc in range(3)]
                    fs += [lambda lt=lt: emit_v(lt)
                           for lt in range(4 * lc, 4 * lc + 4)]
                    return fs

                for f in qkv_fillers(0):
                    f()

                def emit_scores(c, t):
                    """Head-0/1 pair (concurrent K=64 row strips, one 2-bank
                    PSUM tile, one 1024-wide exp) + head-2 solo (K=64, PSUM
                    from the filler pool) for lk tile t.  Scalar-queue order
                    (wide exp first, then head-2) matches the PE-side gates:
                    the single wide buffer is re-written only two Scalar
                    instructions after its exp, and the filler pool's
                    rotation distance of 3 covers the head-2 tiles."""
                    j = t - 4 * c
                    col0 = 128 * j if j >= 0 else 0
                    lq = slice(c * 512 + col0, (c + 1) * 512)
                    tc_ = slice(t * 128, (t + 1) * 128)
                    w = wp.tile([128, 2, 512], F32, tag="w",
                                name=f"w_c{c}t{t}")
                    nc.tensor.matmul(w[:, 0, col0:], kp[0:64, tc_],
                                     qp[0:64, lq])
                    nc.tensor.matmul(w[:, 1, col0:], kp[64:128, tc_],
                                     qp[64:128, lq])
                    ptw = ptwp.tile([128, 2, 512], BF, tag="ptw",
                                    name=f"ptw_c{c}t{t}")
                    nc.scalar.activation(
                        ptw[:, :, col0:], w[:, :, col0:],
                        mybir.ActivationFunctionType.Exp, scale=float(SCALE))
                    s = p1p.tile([128, 512], F32, tag="p1", name=f"s_c{c}t{t}")
                    nc.tensor.matmul(s[:, col0:], k2[0:64, tc_],
                                     q2[0:64, lq])
                    pts = ptsp.tile([128, 512], BF, tag="pts",
                                    name=f"pts_c{c}t{t}")
                    nc.scalar.activation(
                        pts[:, col0:], s[:, col0:],
                        mybir.ActivationFunctionType.Exp, scale=float(SCALE))
                    if j >= 0:
                        # only the 128-wide diagonal band is partial;
                        # columns beyond it are fully valid
                        band = slice(col0, col0 + 128)
                        mb = mask_sb[:, j, band]
                        nc.vector.tensor_mul(ptw[:, 0, band], ptw[:, 0, band], mb)
                        nc.vector.tensor_mul(ptw[:, 1, band], ptw[:, 1, band], mb)
                        nc.vector.tensor_mul(pts[:, band], pts[:, band], mb)
                    return [(0, ptw[:, 0, :], col0, t),
                            (1, ptw[:, 1, :], col0, t),
                            (2, pts, col0, t)]

                # (attnT tile, destination row base) per head
                norm_dest = [(attnT01, 0), (attnT01, 64), (attnT2, 0)]

                def emit_pv_batch(batch):
                    for (h, src, c0, t0, st, sp_) in batch["entries"]:
                        nc.tensor.matmul(
                            batch["pv_acc"][h][:, c0:],
                            vones[:, t0, h * 65:(h + 1) * 65],
                            src[:, c0:],
                            start=st, stop=sp_,
                        )
                    if batch["last"]:
                        emit_norm(batch["c"], batch["pv_acc"])

                def emit_norm(c, pv_acc):
                    rbs_h = []
                    for h in range(HPC):
                        dn = rp.tile([1, 512], F32, tag="dn",
                                     name=f"dn_c{c}h{h}")
                        # partition-shifting copy (psum row 64 -> sbuf row 0);
                        # partition_broadcast only honors a partition-0 source
                        nc.vector.tensor_copy(dn[0:1, :],
                                              pv_acc[h][64:65, :])
                        dnb = rp.tile([64, 512], F32, tag="dnb",
                                      name=f"dnb_c{c}h{h}")
                        nc.gpsimd.partition_broadcast(dnb, dn[0:1, :])
                        rbs = rp.tile([64, 512], F32, tag="rbs",
                                      name=f"rbs_c{c}h{h}")
                        nc.vector.reciprocal_approx_fast(out=rbs, in_=dnb)
                        rbs_h.append(rbs)
                    if c < LC - 1:
                        for h in range(HPC):
                            dt_, r0 = norm_dest[h]
                            nc.vector.tensor_mul(
                                dt_[r0:r0 + 64, c * 512:(c + 1) * 512],
                                pv_acc[h][0:64, :], rbs_h[h],
                            )
                    else:
                        # last chunk: normalise per 128-column piece and kick
                        # its output projection immediately so the epilogue
                        # overlaps DVE/PE/DMA instead of serialising
                        for i in range(4):
                            pc = slice(128 * i, 128 * (i + 1))
                            for h in range(HPC):
                                dt_, r0 = norm_dest[h]
                                nc.vector.tensor_mul(
                                    dt_[r0:r0 + 64,
                                        c * 512 + 128 * i:c * 512 + 128 * (i + 1)],
                                    pv_acc[h][0:64, pc], rbs_h[h][:, pc],
                                )
                            emit_proj(4 * c + i)

                # the PV pipeline runs LAG 2-lk-tile groups behind the
                # scores and flows ACROSS chunk boundaries: a chunk's last
                # PV batches and its ~6us normalisation chain execute under
                # the next chunk's first score groups instead of stalling
                # the PE on the pv-accumulator reuse (WAR on 3 PSUM banks)
                LAG = 3
                pvq = []
                for c in range(LC):
                    nt = 4 * (c + 1)
                    ngroups = nt // 2
                    fillA = qkv_fillers(c + 1) if c + 1 < LC else []
                    # proj fillers for c-1 may only run once norm(c-1) --
                    # emitted at group LAG-1 of this chunk -- is in flight
                    fillB = ([lambda lt=lt: emit_proj(lt)
                              for lt in range(4 * (c - 1), 4 * c)]
                             if c >= 1 else [])
                    pv_acc = [pvp.tile([65, 512], F32, tag="pvacc",
                                       name=f"pvacc_c{c}h{h}")
                              for h in range(HPC)]
                    fa = fb = 0
                    for g in range(ngroups):
                        cur = emit_scores(c, 2 * g)
                        # some PE filler work between the two score tiles
                        # bridges the wide exp of tile t draining before the
                        # single wide PSUM buffer is re-written for t+1
                        want = (2 * g + 1) * len(fillA) // (2 * ngroups)
                        while fa < want:
                            fillA[fa]()
                            fa += 1
                        cur += emit_scores(c, 2 * g + 1)
                        cur = [(h, src, c0, t0, t0 == 0, t0 == nt - 1)
                               for (h, src, c0, t0) in cur]
                        pvq.append({"entries": cur, "c": c, "pv_acc": pv_acc,
                                    "last": g == ngroups - 1})
                        if len(pvq) > LAG:
                            emit_pv_batch(pvq.pop(0))
                        want = (g + 1) * len(fillA) // ngroups
                        while fa < want:
                            fillA[fa]()
                            fa += 1
                        if g >= LAG and fillB:
                            want = (g - LAG + 1) * len(fillB) // (ngroups - LAG)
                            while fb < want:
                                fillB[fb]()
                                fb += 1
                    while fb < len(fillB):
                        fillB[fb]()
                        fb += 1
                for batch in pvq:
                    emit_pv_batch(batch)

    nc.compile()
    return nc


def make_in_maps(x, w_qkv, b_qkv, w_out, L=L_FULL):
    """Host-side sharding: build the 8 per-core input dicts."""
    # causal mask tiles for diagonal blocks: m[p, j, f] = (128 j + p) <= f
    p = np.arange(128)[:, None, None]
    jj = np.arange(4)[None, :, None]
    f = np.arange(512)[None, None, :]
    masks = ((128 * jj + p) <= f).astype(BF16)

    xT = [np.ascontiguousarray(x[b].T.astype(BF16)) for b in range(B)]
    in_maps = []
    for c in range(N_CORES):
        b, g = divmod(c, TPG)
        h0 = g * HPC  # first global head of this group

        def qcol(h):
            return slice((h0 + h) * D_HEAD, (h0 + h + 1) * D_HEAD)

        def kcol(h):
            return slice(768 + (h0 + h) * D_HEAD, 768 + (h0 + h + 1) * D_HEAD)

        wqkc = np.zeros((D_MODEL, 384), np.float32)
        bqkc = np.zeros((384,), np.float32)
        # chunk0 [q0|q1], chunk1 [k0|k1], chunk2 [q2|k2]
        for h in range(2):
            wqkc[:, h * 64:(h + 1) * 64] = w_qkv[:, qcol(h)]
            wqkc[:, 128 + h * 64:128 + (h + 1) * 64] = w_qkv[:, kcol(h)]
            bqkc[h * 64:(h + 1) * 64] = b_qkv[qcol(h)]
            bqkc[128 + h * 64:128 + (h + 1) * 64] = b_qkv[kcol(h)]
        wqkc[:, 256:320] = w_qkv[:, qcol(2)]
        bqkc[256:320] = b_qkv[qcol(2)]
        wqkc[:, 320:384] = w_qkv[:, kcol(2)]
        bqkc[320:384] = b_qkv[kcol(2)]

        wv = w_qkv[:, 1536 + h0 * 64:1536 + (h0 + HPC) * 64]
        wo = w_out[h0 * 64:(h0 + HPC) * 64, :]
        wo3 = np.zeros((128, D_MODEL), np.float32)
        wo3[0:64] = wo[128:192]

        in_maps.append({
            "xT": xT[b][:, :L],
            "wqkc": wqkc.astype(BF16),
            "bqkt": np.ascontiguousarray(bqkc.reshape(3, 128).T),
            "wv": np.ascontiguousarray(wv).astype(BF16),
            "wo2": np.ascontiguousarray(wo[0:128]).astype(BF16),
            "wo3": wo3.astype(BF16),
            "masks": masks,
        })
    return in_maps


_NC_CACHE = {}


def _get_nc(L=L_FULL):
    if L not in _NC_CACHE:
        _NC_CACHE[L] = build_nc(L)
    return _NC_CACHE[L]


def run(x, w_qkv, b_qkv, w_out, b_out, L=L_FULL, trace=False):
    nc = _get_nc(L)
    in_maps = make_in_maps(np.asarray(x), np.asarray(w_qkv),
                           np.asarray(b_qkv), np.asarray(w_out), L=L)
    if trace:
        install_ntff()
    res = run_bass_kernel_spmd(nc, in_maps, core_ids=list(range(N_CORES)),
                               trace=trace)
    partials = np.stack([res.results[c]["out"] for c in range(N_CORES)])
    out = partials.reshape(B, TPG, L, D_MODEL).sum(axis=1)
    # the V bias commutes through the attention average (weights sum to 1),
    # so it collapses to a constant row applied after the projection
    bias = np.asarray(b_qkv, np.float32)[1536:] @ np.asarray(w_out, np.float32)
    out = out + (bias + np.asarray(b_out, np.float32))[None, None, :]
    return out.astype(np.float32), res


def kernel(x, w_qkv, b_qkv, w_out, b_out):
    out, _ = run(x, w_qkv, b_qkv, w_out, b_out, L=L_FULL, trace=False)
    return out


# ---- optional NTFF profiling hook (axon images lack antenv.axon_hooks) ----
def install_ntff(so_path="/opt/axon/libaxon_pjrt.so"):
    import contextlib
    import ctypes
    import types

    if "antenv.axon_hooks" in sys.modules:
        return
    holder = {"hook": None}

    def _build():
        if not os.path.exists(so_path):
            return None
        lib = ctypes.CDLL(so_path)
        if not hasattr(lib, "axon_start_nrt_profile"):
            return None
        lib.axon_start_nrt_profile.argtypes = [ctypes.POINTER(ctypes.c_int64),
                                               ctypes.c_size_t]
        lib.axon_start_nrt_profile.restype = ctypes.c_int64
        lib.axon_stop_nrt_profile.argtypes = [ctypes.c_char_p]
        lib.axon_stop_nrt_profile.restype = ctypes.c_int64

        @contextlib.contextmanager
        def _hook(output_dir, device_ids):
            import jax
            jax.devices()
            if device_ids:
                ids = (ctypes.c_int64 * len(device_ids))(*device_ids)
                rc = lib.axon_start_nrt_profile(ids, len(device_ids))
            else:
                rc = lib.axon_start_nrt_profile(None, 0)
            if rc != 0:
                raise RuntimeError(f"axon_start_nrt_profile rc={rc}")
            try:
                yield
            finally:
                n = lib.axon_stop_nrt_profile(str(output_dir).encode())
                print(f"ntff profile: {n} file(s) -> {output_dir}",
                      file=sys.stderr)

        return _hook

    mod = types.ModuleType("antenv.axon_hooks")
    mod.set_axon_ntff_profile_hook = lambda h: holder.__setitem__("hook", h)
    mod.get_axon_ntff_profile_hook = lambda: holder["hook"]
    sys.modules["antenv.axon_hooks"] = mod
    holder["hook"] = _build()


# revision 13
# speedup vs baseline: 1.0145x; 1.0078x over previous
"""Causal self-attention (B=2, L=4096, D=768, H=12) on 8 Trainium2 cores.

Sharding: core c = (b, g) with b = c // 4, g = c % 4. Data-parallel over the
batch, tensor-parallel over heads (3 heads per core). Each core computes its
heads' attention over the full sequence plus its slice of the output
projection (row-parallel); the host sums the 4 partial projections per batch
element and adds b_out.

Device-side design (all matmul operands bf16, fp32 PSUM accumulation):
  - host supplies x[b]^T (768, L); q^T/k^T are produced in [dh, L] layout by
    the projection itself, v in natural [L, dh] layout — no on-device
    transposes anywhere.
  - heads 0/1 are PACKED on partitions: qp/kp [128, L] hold head0 in rows
    0-63 and head1 in rows 64-127.  Their score matmuls are K=64 with
    tile_position row strips (0,0)/(64,0) and run CONCURRENTLY on the PE
    (two 64-row sub-arrays), halving score time for those heads.  Head 2
    (q2/k2, rows 0-63 only) runs K=64 solo on strips 0-1, overlapping the
    tail of head1's stream.
  - the head-0/1 score pair lands in one 2-bank-wide PSUM tile, so a SINGLE
    ScalarE exp covers 1024 columns — activation instruction count (and the
    per-instruction (112 cyc)/semaphore overhead on the bottleneck ScalarE)
    drops ~40%.  Scores are computed transposed, S^T[lk, lq], 512 lq per
    chunk; exp runs with the 1/sqrt(dh) scale folded in and no
    max-subtraction (scores are ~N(0,1) here, exp is safe in fp32).
  - causal mask = multiply by a 0/1 tile, needed only on the 4 diagonal
    lk-tiles of each lq chunk.
  - PV matmul uses lhsT = [v | ones] so PSUM row 64 accumulates the softmax
    denominator for free; the reciprocal row is partition-broadcast on the
    (otherwise idle) GpSimd engine and applied by one DVE multiply.
  - PSUM budget (8 banks): score-pair wide tile 2 + head-2 single 1 +
    pv accumulators 3 + filler (qkv/out-proj) pool 2.
  - the PE clock-gate (HAM) only sustains 2.4 GHz when the PE never idles,
    so the qkv projection of the NEXT lq chunk and the output projection of
    the PREVIOUS chunk are emitted as filler work inside the attention
    loop, and PV runs software-pipelined one 2-lk-tile group behind the exp.
"""

import os
import sys

sys.path.insert(0, "/opt/trn_rl_repo")

import numpy as np
import ml_dtypes

import concourse.bass as bass  # noqa: F401  (registers AP machinery)
import concourse.mybir as mybir
from concourse import bacc
import concourse.tile as tile
from concourse.bass_utils import run_bass_kernel_spmd

BF16 = ml_dtypes.bfloat16
F32 = mybir.dt.float32
BF = mybir.dt.bfloat16

D_MODEL = 768
N_HEADS = 12
D_HEAD = 64
B = 2
L_FULL = 4096
N_CORES = 8
TPG = 4  # head-groups (tensor-parallel degree per batch element)
HPC = N_HEADS // TPG  # 3 heads per core
DG = HPC * D_HEAD  # 192 feature dims per core
SCALE = 1.0 / np.sqrt(D_HEAD)

DM_CHUNKS = D_MODEL // 128  # 6


def build_nc(L=L_FULL):
    """Build the per-core Bass program (same program for all 8 cores)."""
    LC = L // 512  # lq chunks
    LT = L // 128  # lk / l tiles
    nc = bacc.Bacc("TRN2", target_bir_lowering=False, debug=False,
                   num_devices=N_CORES)

    xT_d = nc.dram_tensor("xT", [D_MODEL, L], BF, kind="ExternalInput").ap()
    wqk_d = nc.dram_tensor("wqkc", [D_MODEL, 384], BF, kind="ExternalInput").ap()
    bqk_d = nc.dram_tensor("bqkt", [128, 3], F32, kind="ExternalInput").ap()
    wv_d = nc.dram_tensor("wv", [D_MODEL, DG], BF, kind="ExternalInput").ap()
    wo2_d = nc.dram_tensor("wo2", [128, D_MODEL], BF, kind="ExternalInput").ap()
    wo3_d = nc.dram_tensor("wo3", [128, D_MODEL], BF, kind="ExternalInput").ap()
    mask_d = nc.dram_tensor("masks", [128, 4, 512], BF, kind="ExternalInput").ap()
    out_d = nc.dram_tensor("out", [L, D_MODEL], F32, kind="ExternalOutput").ap()

    with tile.TileContext(nc) as tc:
        with tc.tile_pool(name="persist", bufs=1) as persist:
            xT_sb = persist.tile([128, DM_CHUNKS, L], BF)
            wqk_sb = persist.tile([128, DM_CHUNKS, 384], BF)
            wv_sb = persist.tile([128, DM_CHUNKS, DG], BF)
            bqk_sb = persist.tile([128, 3], F32)
            wo2_sb = persist.tile([128, D_MODEL], BF)
            wo3_sb = persist.tile([128, D_MODEL], BF)
            mask_sb = persist.tile([128, 4, 512], BF)
            # heads 0/1 packed on partitions (rows 0-63 / 64-127); head 2
            # lives alone in rows 0-63 of its own tiles -> K=64 matmuls, no
            # zero padding anywhere
            qp = persist.tile([128, L], BF, name="qp01")
            kp = persist.tile([128, L], BF, name="kp01")
            # head 2 is duplicated in rows 64-127 so consecutive lk tiles can
            # run as a concurrent (0,0)/(64,0) row-strip pair like heads 0/1
            q2 = persist.tile([128, L], BF, name="q2")
            k2 = persist.tile([128, L], BF, name="k2")
            vones = persist.tile([128, LT, HPC * 65], BF)
            attnT01 = persist.tile([128, L], BF)
            attnT2 = persist.tile([128, L], BF)

            # warmup scratch: garbage matmuls during the initial DMA keep the
            # PE busy so the HAM clock-gate is already at 2.4 GHz when real
            # work arrives
            scratch = persist.tile([128, 512], BF, name="scratch")

            # only the interleaved ones-columns of vones need initialising
            # (the data columns are fully written by the v evacuations); the
            # proj reads attnT2 rows 64-127 as zero K-padding
            nc.vector.memset(scratch, 0.0)
            nc.vector.memset(vones[:, :, 64:HPC * 65:65], 1.0)
            nc.gpsimd.memset(attnT2[64:128, :], 0.0)
            # x^T in lq-ordered strips so chunk 0's projections start early;
            # pair each weight chunk with its 512-wide first strip so fc=0's
            # accumulation chain unblocks as soon as possible
            strips = [(0, 512), (512, 1024)]
            strips += [(s, s + 1024) for s in range(1024, L, 1024)]
            for cdm in range(DM_CHUNKS):
                nc.sync.dma_start(out=wqk_sb[:, cdm, :],
                                  in_=wqk_d[cdm * 128:(cdm + 1) * 128, :])
                nc.sync.dma_start(
                    out=xT_sb[:, cdm, 0:512],
                    in_=xT_d[cdm * 128:(cdm + 1) * 128, 0:512])
            nc.sync.dma_start(out=mask_sb, in_=mask_d)
            nc.sync.dma_start(out=bqk_sb, in_=bqk_d)
            for s0, s1 in strips[1:2]:
                for cdm in range(DM_CHUNKS):
                    nc.sync.dma_start(
                        out=xT_sb[:, cdm, s0:s1],
                        in_=xT_d[cdm * 128:(cdm + 1) * 128, s0:s1])
            for cdm in range(DM_CHUNKS):
                nc.sync.dma_start(out=wv_sb[:, cdm, :],
                                  in_=wv_d[cdm * 128:(cdm + 1) * 128, :])
            nc.sync.dma_start(out=wo2_sb, in_=wo2_d)
            nc.sync.dma_start(out=wo3_sb, in_=wo3_d)
            for s0, s1 in strips[2:]:
                for cdm in range(DM_CHUNKS):
                    nc.sync.dma_start(
                        out=xT_sb[:, cdm, s0:s1],
                        in_=xT_d[cdm * 128:(cdm + 1) * 128, s0:s1])

            # wqkc column chunks: 0=[q0|q1] 1=[k0|k1] 2=[q2|k2]; chunks 0/1
            # evacuate whole (partition layout already matches qp/kp), chunk
            # 2 splits rows 0-63 -> q2, rows 64-127 -> k2 (shifted copy)
            with (
                tc.tile_pool(name="p1psum", bufs=3, space="PSUM") as p1p,
                tc.tile_pool(name="wpsum", bufs=1, space="PSUM") as wp,
                tc.tile_pool(name="pvpsum", bufs=3, space="PSUM") as pvp,
                tc.tile_pool(name="ptwpool", bufs=8) as ptwp,
                tc.tile_pool(name="ptspool", bufs=8) as ptsp,
                tc.tile_pool(name="rpool", bufs=3) as rp,
                tc.tile_pool(name="outpool", bufs=3) as outp,
            ):
                # PE warmup during the input DMA (see scratch above)
                for i in range(26):
                    pw = p1p.tile([128, 512], F32, tag="p1", name=f"warm{i}")
                    nc.tensor.matmul(pw, scratch[:, 0:128], scratch,
                                     start=True, stop=True)
                def emit_qk(fc, lc):
                    ps = p1p.tile([128, 512], F32, tag="p1",
                                  name=f"psqk{fc}_{lc}")
                    for cdm in range(DM_CHUNKS):
                        nc.tensor.matmul(
                            ps,
                            wqk_sb[:, cdm, fc * 128:(fc + 1) * 128],
                            xT_sb[:, cdm, lc * 512:(lc + 1) * 512],
                            start=(cdm == 0), stop=(cdm == DM_CHUNKS - 1),
                        )
                    lq = slice(lc * 512, (lc + 1) * 512)
                    if fc == 0:
                        nc.vector.tensor_scalar_add(
                            qp[:, lq], ps, bqk_sb[:, 0:1])
                    elif fc == 1:
                        nc.vector.tensor_scalar_add(
                            kp[:, lq], ps, bqk_sb[:, 1:2])
                    else:
                        nc.vector.tensor_scalar_add(
                            q2[0:64, lq], ps[0:64, :], bqk_sb[0:64, 2:3])
                        nc.vector.tensor_scalar_add(
                            q2[64:128, lq], ps[0:64, :], bqk_sb[0:64, 2:3])
                        nc.vector.tensor_scalar_add(
                            k2[0:64, lq], ps[64:128, :], bqk_sb[64:128, 2:3])
                        nc.vector.tensor_scalar_add(
                            k2[64:128, lq], ps[64:128, :], bqk_sb[64:128, 2:3])

                def emit_v(lt):
                    ps = p1p.tile([128, DG], F32, tag="p1", name=f"psv{lt}")
                    for cdm in range(DM_CHUNKS):
                        nc.tensor.matmul(
                            ps,
                            xT_sb[:, cdm, lt * 128:(lt + 1) * 128],
                            wv_sb[:, cdm, :],
                            start=(cdm == 0), stop=(cdm == DM_CHUNKS - 1),
                        )
                    nc.vector.tensor_copy(
                        vones[:, lt, 0:HPC * 65]
                        .rearrange("p (h c) -> p h c", h=HPC)[:, :, 0:64],
                        ps.rearrange("p (h c) -> p h c", h=HPC),
                    )

                def emit_proj(lt):
                    osb = outp.tile([128, D_MODEL], F32, tag="osb",
                                    name=f"osb{lt}")
                    for nh in range(2):
                        po = p1p.tile([128, 384], F32, tag="p1",
                                      name=f"po{lt}_{nh}")
                        nc.tensor.matmul(
                            po,
                            attnT01[:, lt * 128:(lt + 1) * 128],
                            wo2_sb[:, nh * 384:(nh + 1) * 384],
                            start=True, stop=False,
                        )
                        nc.tensor.matmul(
                            po,
                            attnT2[:, lt * 128:(lt + 1) * 128],
                            wo3_sb[:, nh * 384:(nh + 1) * 384],
                            start=False, stop=True,
                        )
                        nc.vector.tensor_copy(osb[:, nh * 384:(nh + 1) * 384], po)
                    nc.sync.dma_start(out=out_d[lt * 128:(lt + 1) * 128, :],
                                      in_=osb)

                def qkv_fillers(lc):
                    fs = [lambda fc=fc: emit_qk(fc, lc) for fc in range(3)]
                    fs += [lambda lt=lt: emit_v(lt)
                           for lt in range(4 * lc, 4 * lc + 4)]
                    return fs

                for f in qkv_fillers(0):
                    f()

                def emit_scores_w(c, t):
                    """Head-0/1 pair for lk tile t: concurrent K=64 row
                    strips (0,0)/(64,0), one 2-bank PSUM tile, one 1024-wide
                    exp."""
                    j = t - 4 * c
                    col0 = 128 * j if j >= 0 else 0
                    lq = slice(c * 512 + col0, (c + 1) * 512)
                    tc_ = slice(t * 128, (t + 1) * 128)
                    w = wp.tile([128, 2, 512], F32, tag="w",
                                name=f"w_c{c}t{t}")
                    nc.tensor.matmul(w[:, 0, col0:], kp[0:64, tc_],
                                     qp[0:64, lq])
                    nc.tensor.matmul(w[:, 1, col0:], kp[64:128, tc_],
                                     qp[64:128, lq])
                    ptw = ptwp.tile([128, 2, 512], BF, tag="ptw",
                                    name=f"ptw_c{c}t{t}")
                    nc.scalar.activation(
                        ptw[:, :, col0:], w[:, :, col0:],
                        mybir.ActivationFunctionType.Exp, scale=float(SCALE))
                    if j >= 0:
                        # only the 128-wide diagonal band is partial;
                        # columns beyond it are fully valid
                        band = slice(col0, col0 + 128)
                        mb = mask_sb[:, j, band]
                        nc.vector.tensor_mul(ptw[:, 0, band], ptw[:, 0, band], mb)
                        nc.vector.tensor_mul(ptw[:, 1, band], ptw[:, 1, band], mb)
                    return [(0, ptw[:, 0, :], col0, t),
                            (1, ptw[:, 1, :], col0, t)]

                def emit_scores_h2(c, t, t2):
                    """Head 2 for lk tiles t/t2 as a concurrent row-strip
                    pair (via the rows-64..127 duplicate of q2/k2); the two
                    PSUM tiles are consecutive filler-pool tenants and so
                    always land in different banks."""
                    out = []
                    for base, tt in ((0, t), (64, t2)):
                        j = tt - 4 * c
                        col0 = 128 * j if j >= 0 else 0
                        lq = slice(c * 512 + col0, (c + 1) * 512)
                        tc_ = slice(tt * 128, (tt + 1) * 128)
                        s = p1p.tile([128, 512], F32, tag="p1",
                                     name=f"s_c{c}t{tt}")
                        nc.tensor.matmul(s[:, col0:],
                                         k2[base:base + 64, tc_],
                                         q2[base:base + 64, lq])
                        pts = ptsp.tile([128, 512], BF, tag="pts",
                                        name=f"pts_c{c}t{tt}")
                        nc.scalar.activation(
                            pts[:, col0:], s[:, col0:],
                            mybir.ActivationFunctionType.Exp,
                            scale=float(SCALE))
                        if j >= 0:
                            band = slice(col0, col0 + 128)
                            nc.vector.tensor_mul(pts[:, band], pts[:, band],
                                                 mask_sb[:, j, band])
                        out.append((2, pts, col0, tt))
                    return out

                # (attnT tile, destination row base) per head
                norm_dest = [(attnT01, 0), (attnT01, 64), (attnT2, 0)]

                def emit_pv_batch(batch):
                    for (h, src, c0, t0, st, sp_) in batch["entries"]:
                        nc.tensor.matmul(
                            batch["pv_acc"][h][:, c0:],
                            vones[:, t0, h * 65:(h + 1) * 65],
                            src[:, c0:],
                            start=st, stop=sp_,
                        )
                    if batch["last"]:
                        emit_norm(batch["c"], batch["pv_acc"])

                def emit_norm(c, pv_acc):
                    rbs_h = []
                    for h in range(HPC):
                        dn = rp.tile([1, 512], F32, tag="dn",
                                     name=f"dn_c{c}h{h}")
                        # partition-shifting copy (psum row 64 -> sbuf row 0);
                        # partition_broadcast only honors a partition-0 source
                        nc.vector.tensor_copy(dn[0:1, :],
                                              pv_acc[h][64:65, :])
                        dnb = rp.tile([64, 512], F32, tag="dnb",
                                      name=f"dnb_c{c}h{h}")
                        nc.gpsimd.partition_broadcast(dnb, dn[0:1, :])
                        rbs = rp.tile([64, 512], F32, tag="rbs",
                                      name=f"rbs_c{c}h{h}")
                        nc.vector.reciprocal_approx_fast(out=rbs, in_=dnb)
                        rbs_h.append(rbs)
                    if c < LC - 1:
                        for h in range(HPC):
                            dt_, r0 = norm_dest[h]
                            nc.vector.tensor_mul(
                                dt_[r0:r0 + 64, c * 512:(c + 1) * 512],
                                pv_acc[h][0:64, :], rbs_h[h],
                            )
                    else:
                        # last chunk: normalise per 128-column piece and kick
                        # its output projection immediately so the epilogue
                        # overlaps DVE/PE/DMA instead of serialising
                        for i in range(4):
                            pc = slice(128 * i, 128 * (i + 1))
                            for h in range(HPC):
                                dt_, r0 = norm_dest[h]
                                nc.vector.tensor_mul(
                                    dt_[r0:r0 + 64,
                                        c * 512 + 128 * i:c * 512 + 128 * (i + 1)],
                                    pv_acc[h][0:64, pc], rbs_h[h][:, pc],
                                )
                            emit_proj(4 * c + i)

                # the PV pipeline runs LAG 2-lk-tile groups behind the
                # scores and flows ACROSS chunk boundaries: a chunk's last
                # PV batches and its ~6us normalisation chain execute under
                # the next chunk's first score groups instead of stalling
                # the PE on the pv-accumulator reuse (WAR on 3 PSUM banks)
                LAG = 3
                pvq = []
                for c in range(LC):
                    nt = 4 * (c + 1)
                    ngroups = nt // 2
                    fillA = qkv_fillers(c + 1) if c + 1 < LC else []
                    # proj fillers for c-1 may only run once norm(c-1) --
                    # emitted at group LAG-1 of this chunk -- is in flight
                    fillB = ([lambda lt=lt: emit_proj(lt)
                              for lt in range(4 * (c - 1), 4 * c)]
                             if c >= 1 else [])
                    pv_acc = [pvp.tile([65, 512], F32, tag="pvacc",
                                       name=f"pvacc_c{c}h{h}")
                              for h in range(HPC)]
                    fa = fb = 0
                    for g in range(ngroups):
                        cur = emit_scores_w(c, 2 * g)
                        # the PV batch sits between the two W pairs: ready
                        # PE work (its pt inputs are LAG groups old) that
                        # bridges the wide exp of tile t draining before the
                        # single wide PSUM buffer is re-written for t+1 --
                        # the PE is in-order, so gated scores must never sit
                        # ahead of ready work in the queue
                        if len(pvq) >= LAG:
                            emit_pv_batch(pvq.pop(0))
                        want = (2 * g + 1) * len(fillA) // (2 * ngroups)
                        while fa < want:
                            fillA[fa]()
                            fa += 1
                        cur += emit_scores_w(c, 2 * g + 1)
                        cur += emit_scores_h2(c, 2 * g, 2 * g + 1)
                        cur = [(h, src, c0, t0, t0 == 0, t0 == nt - 1)
                               for (h, src, c0, t0) in cur]
                        pvq.append({"entries": cur, "c": c, "pv_acc": pv_acc,
                                    "last": g == ngroups - 1})
                        want = (g + 1) * len(fillA) // ngroups
                        while fa < want:
                            fillA[fa]()
                            fa += 1
                        if g >= LAG and fillB:
                            want = (g - LAG + 1) * len(fillB) // (ngroups - LAG)
                            while fb < want:
                                fillB[fb]()
                                fb += 1
                    while fb < len(fillB):
                        fillB[fb]()
                        fb += 1
                for batch in pvq:
                    emit_pv_batch(batch)

    nc.compile()
    return nc


def make_in_maps(x, w_qkv, b_qkv, w_out, L=L_FULL):
    """Host-side sharding: build the 8 per-core input dicts."""
    # causal mask tiles for diagonal blocks: m[p, j, f] = (128 j + p) <= f
    p = np.arange(128)[:, None, None]
    jj = np.arange(4)[None, :, None]
    f = np.arange(512)[None, None, :]
    masks = ((128 * jj + p) <= f).astype(BF16)

    xT = [np.ascontiguousarray(x[b].T.astype(BF16)) for b in range(B)]
    in_maps = []
    for c in range(N_CORES):
        b, g = divmod(c, TPG)
        h0 = g * HPC  # first global head of this group

        def qcol(h):
            return slice((h0 + h) * D_HEAD, (h0 + h + 1) * D_HEAD)

        def kcol(h):
            return slice(768 + (h0 + h) * D_HEAD, 768 + (h0 + h + 1) * D_HEAD)

        wqkc = np.zeros((D_MODEL, 384), np.float32)
        bqkc = np.zeros((384,), np.float32)
        # chunk0 [q0|q1], chunk1 [k0|k1], chunk2 [q2|k2]
        for h in range(2):
            wqkc[:, h * 64:(h + 1) * 64] = w_qkv[:, qcol(h)]
            wqkc[:, 128 + h * 64:128 + (h + 1) * 64] = w_qkv[:, kcol(h)]
            bqkc[h * 64:(h + 1) * 64] = b_qkv[qcol(h)]
            bqkc[128 + h * 64:128 + (h + 1) * 64] = b_qkv[kcol(h)]
        wqkc[:, 256:320] = w_qkv[:, qcol(2)]
        bqkc[256:320] = b_qkv[qcol(2)]
        wqkc[:, 320:384] = w_qkv[:, kcol(2)]
        bqkc[320:384] = b_qkv[kcol(2)]

        wv = w_qkv[:, 1536 + h0 * 64:1536 + (h0 + HPC) * 64]
        wo = w_out[h0 * 64:(h0 + HPC) * 64, :]
        wo3 = np.zeros((128, D_MODEL), np.float32)
        wo3[0:64] = wo[128:192]

        in_maps.append({
            "xT": xT[b][:, :L],
            "wqkc": wqkc.astype(BF16),
            "bqkt": np.ascontiguousarray(bqkc.reshape(3, 128).T),
            "wv": np.ascontiguousarray(wv).astype(BF16),
            "wo2": np.ascontiguousarray(wo[0:128]).astype(BF16),
            "wo3": wo3.astype(BF16),
            "masks": masks,
        })
    return in_maps


_NC_CACHE = {}


def _get_nc(L=L_FULL):
    if L not in _NC_CACHE:
        _NC_CACHE[L] = build_nc(L)
    return _NC_CACHE[L]


def run(x, w_qkv, b_qkv, w_out, b_out, L=L_FULL, trace=False):
    nc = _get_nc(L)
    in_maps = make_in_maps(np.asarray(x), np.asarray(w_qkv),
                           np.asarray(b_qkv), np.asarray(w_out), L=L)
    if trace:
        install_ntff()
    res = run_bass_kernel_spmd(nc, in_maps, core_ids=list(range(N_CORES)),
                               trace=trace)
    partials = np.stack([res.results[c]["out"] for c in range(N_CORES)])
    out = partials.reshape(B, TPG, L, D_MODEL).sum(axis=1)
    # the V bias commutes through the attention average (weights sum to 1),
    # so it collapses to a constant row applied after the projection
    bias = np.asarray(b_qkv, np.float32)[1536:] @ np.asarray(w_out, np.float32)
    out = out + (bias + np.asarray(b_out, np.float32))[None, None, :]
    return out.astype(np.float32), res


def kernel(x, w_qkv, b_qkv, w_out, b_out):
    out, _ = run(x, w_qkv, b_qkv, w_out, b_out, L=L_FULL, trace=False)
    return out


# ---- optional NTFF profiling hook (axon images lack antenv.axon_hooks) ----
def install_ntff(so_path="/opt/axon/libaxon_pjrt.so"):
    import contextlib
    import ctypes
    import types

    if "antenv.axon_hooks" in sys.modules:
        return
    holder = {"hook": None}

    def _build():
        if not os.path.exists(so_path):
            return None
        lib = ctypes.CDLL(so_path)
        if not hasattr(lib, "axon_start_nrt_profile"):
            return None
        lib.axon_start_nrt_profile.argtypes = [ctypes.POINTER(ctypes.c_int64),
                                               ctypes.c_size_t]
        lib.axon_start_nrt_profile.restype = ctypes.c_int64
        lib.axon_stop_nrt_profile.argtypes = [ctypes.c_char_p]
        lib.axon_stop_nrt_profile.restype = ctypes.c_int64

        @contextlib.contextmanager
        def _hook(output_dir, device_ids):
            import jax
            jax.devices()
            if device_ids:
                ids = (ctypes.c_int64 * len(device_ids))(*device_ids)
                rc = lib.axon_start_nrt_profile(ids, len(device_ids))
            else:
                rc = lib.axon_start_nrt_profile(None, 0)
            if rc != 0:
                raise RuntimeError(f"axon_start_nrt_profile rc={rc}")
            try:
                yield
            finally:
                n = lib.axon_stop_nrt_profile(str(output_dir).encode())
                print(f"ntff profile: {n} file(s) -> {output_dir}",
                      file=sys.stderr)

        return _hook

    mod = types.ModuleType("antenv.axon_hooks")
    mod.set_axon_ntff_profile_hook = lambda h: holder.__setitem__("hook", h)
    mod.get_axon_ntff_profile_hook = lambda: holder["hook"]
    sys.modules["antenv.axon_hooks"] = mod
    holder["hook"] = _build()
